# revision 8
# baseline (speedup 1.0000x reference)
"""Pairwise cosine-similarity kernel for Trainium2 (8 NeuronCores, SPMD).

Computes out = 16 * normalize(x1) @ normalize(x2).T for x1, x2 [8192, 512] f32.

Sharding: x1 rows are split across the 8 cores (1024 rows each); x2 is
replicated. Each core computes its [1024, 8192] slice of the output; the host
concatenates the slices.

Host-side prep is layout/dtype only: inputs are cast to bf16; x1 and x2 are
additionally shipped pre-transposed so neither GEMM operand needs on-device
transposition. All FLOPs (norms, normalization, GEMM, scaling) run on device:

  1. x1 row norms from the natural-layout bf16 shard: fused Square+row-sum on
     ScalarE -> sqrt -> clamp -> reciprocal -> inv1 [128, 8] compact
     (col m <-> output row-tile m). The 16/n1 scale is folded into the
     PSUM->SBUF copy of each output tile (per-partition scale), so x1 goes
     stationary into the PE untouched.
  2. x2 is processed in column blocks straight from the transposed tiles
     (no natural-layout copy is shipped): per block, square/add chains on
     DVE+GpSimd+ACT (f16) form partial sums of squares; one ones-matmul per
     512-chunk both reduces over the 4 K-partitions and partition-broadcasts
     the result; reciprocal (DVE) + Sqrt (ACT) give 1/norm broadcast, and the
     x2T block is scaled in place (16-bit tensor_tensor on DVE/GpSimd).
  3. Main GEMM per (block, m): out_psum[j][128, 512] += x1T_k_tile.T @ x2S
     over 4 K-chunks with up to 4 moving tiles per stationary load (weight
     reuse window for LDWEIGHTS prefetch), bf16 in, f32 PSUM. PSUM->SBUF
     copies are split across DVE/ACT with the x1 row-scale fused; the SBUF
     staging tile is bf16 (halves output DMA); host casts back to f32.

Next-block prep is emitted interleaved between the current block's m-groups
(per-engine queues are strict FIFO; bulk-emitting prep ahead of pending PSUM
copies would stall the copies behind it).
"""

import sys

for _p in ("/root/.axon_site/_ro/trn_rl_repo", "/opt/trn_rl_repo"):
    if _p not in sys.path:
        sys.path.append(_p)

import ml_dtypes
import numpy as np

import concourse.bass as bass
import concourse.tile as tile
from concourse import bacc, mybir
from concourse.bass_utils import run_bass_kernel_spmd

F32 = mybir.dt.float32
F16 = mybir.dt.float16
BF16 = mybir.dt.bfloat16
P = 128
SCALE = 16.0
EPS = 1e-8

N_CORES = 8
N1 = 8192  # x1 rows (total)
N2 = 8192  # x2 rows
D = 512  # feature dim

# x2 column blocks (rows of x2 == columns of out). Lead blocks are small so
# the first GEMM starts early; later blocks are 2048 (4 PSUM banks).
BLOCKS = (512, 512, 1024, 2048, 2048, 2048)
assert sum(BLOCKS) == N2

_PROGRAM_CACHE = {}


def build_program(n1_local=N1 // N_CORES, n2=N2, d=D):
    kc = d // P  # K-chunks of the contraction dim (4)
    m_tiles = n1_local // P  # x1 row-tiles per core (8)

    nc = bacc.Bacc("TRN2", target_bir_lowering=False, debug=False,
                   num_devices=N_CORES)
    x1 = nc.dram_tensor("x1", [n1_local, d], BF16, kind="ExternalInput")
    x1t = nc.dram_tensor("x1t", [d, n1_local], BF16, kind="ExternalInput")
    x2t = nc.dram_tensor("x2t", [d, n2], BF16, kind="ExternalInput")
    out = nc.dram_tensor("out", [n1_local, n2], BF16, kind="ExternalOutput")

    with tile.TileContext(nc) as tc:
        with (
            tc.tile_pool(name="const", bufs=1) as const,
            tc.tile_pool(name="ld", bufs=2) as ld,
            tc.tile_pool(name="sq", bufs=2) as sqp,
            tc.tile_pool(name="stat", bufs=4) as stat,
            tc.tile_pool(name="xt", bufs=1) as xt,
            tc.tile_pool(name="bc", bufs=2) as bcp,
            tc.tile_pool(name="outp", bufs=3) as outp,
            tc.tile_pool(name="ps", bufs=8, space="PSUM") as psp,
        ):
            ones128 = const.tile([P, P], F16)
            nc.gpsimd.memset(ones128[:], 1.0)
            warm = stat.tile([P, 1], F32, tag="warm")

            x1r = x1.ap().rearrange("(g j p) e -> g p j e", j=4, p=P)
            x2t_r = x2t.ap().rearrange("(k p) n -> p k n", p=P)

            # stationary operand: x1T [128, kc, n1_local]
            x1T = xt.tile([P, kc, n1_local], BF16, name="x1T")
            nc.sync.dma_start(
                x1T[:], x1t.ap().rearrange("(k p) n -> p k n", p=P)
            )

            # moving operand per block [128, kc, w], scaled in place
            x2S = [xt.tile([P, kc, w], BF16, tag=f"x2S_{b}", name=f"x2S_{b}")
                   for b, w in enumerate(BLOCKS)]

            # ---- x1 stats: compact inverse norms, scaled by 16 ----------
            inv1 = stat.tile([P, m_tiles], F32, name="inv1")

            def prep_x1():
                for g in range(m_tiles // 4):
                    ld_t = ld.tile([P, 4, d], BF16, tag="ld")
                    nc.gpsimd.dma_start(ld_t[:], x1r[g])
                    ssq = stat.tile([P, 4], F32, tag="ssq")
                    for j in range(4):
                        sq_t = sqp.tile([P, d], BF16, tag="sq")
                        nc.scalar.activation(
                            sq_t[:], ld_t[:, j],
                            mybir.ActivationFunctionType.Square,
                            accum_out=ssq[:, j : j + 1],
                        )
                    nrm = stat.tile([P, 4], F32, tag="nrm")
                    nc.scalar.activation(
                        nrm[:], ssq[:], mybir.ActivationFunctionType.Sqrt
                    )
                    dst = inv1[:, g * 4 : (g + 1) * 4]
                    nc.vector.tensor_scalar_max(nrm[:], nrm[:], EPS)
                    nc.vector.reciprocal(dst, nrm[:])
                    nc.vector.tensor_scalar_mul(dst, dst, SCALE)

            # ---- x2 per block: dma -> sumsq -> reduce+bcast -> scale ----
            block_c0 = [sum(BLOCKS[:b]) for b in range(len(BLOCKS))]

            def prep_block_steps(b, dma_engine=None):
                w = BLOCKS[b]
                c0 = block_c0[b]
                xb = x2S[b]
                t0 = sqp.tile([P, w], F16, tag="t0", bufs=2, name=f"t0_{b}")
                t1 = sqp.tile([P, w], F16, tag="t1", bufs=2, name=f"t1_{b}")
                bc = bcp.tile([P, w], F16, tag="bc", name=f"bc_{b}")
                eng = dma_engine or nc.sync
                steps = [
                    lambda: eng.dma_start(xb[:], x2t_r[:, :, c0 : c0 + w]),
                    # squares (f16) spread over DVE / GpSimd / ACT
                    lambda: nc.vector.tensor_mul(t0[:], xb[:, 0], xb[:, 0]),
                    lambda: nc.gpsimd.tensor_mul(t1[:], xb[:, 1], xb[:, 1]),
                    lambda: (
                        nc.scalar.activation(
                            warm2[:, :w], xb[:, 2],
                            mybir.ActivationFunctionType.Square),
                        nc.vector.tensor_add(t0[:], t0[:], warm2[:, :w]),
                    ),
                    lambda: (
                        nc.gpsimd.tensor_mul(t1b[:, :w], xb[:, 3], xb[:, 3]),
                        nc.gpsimd.tensor_add(t1[:], t1[:], t1b[:, :w]),
                    ),
                    lambda: nc.vector.tensor_add(t0[:], t0[:], t1[:]),
                ]

                # per 512-chunk: reduce over partitions + broadcast via
                # ones-matmul, then reciprocal into bc
                def red(c):
                    nm = psp.tile([P, 512], F32, tag="ps",
                                  name=f"nm_{b}_{c}")
                    nc.tensor.matmul(
                        nm[:], lhsT=ones128[:],
                        rhs=t0[:, c * 512 : (c + 1) * 512],
                        start=True, stop=True,
                    )
                    with nc.allow_low_precision(
                        reason="1/sumsq in f16: values ~2e-3, 11-bit "
                               "mantissa ample for a 2e-2 rel-err budget"
                    ):
                        nc.vector.reciprocal(
                            bc[:, c * 512 : (c + 1) * 512], nm[:]
                        )

                for c in range(w // 512):
                    steps.append(lambda c=c: red(c))
                # bc = sqrt(1/sumsq) = 1/norm
                steps.append(lambda: nc.scalar.activation(
                    bc[:], bc[:], mybir.ActivationFunctionType.Sqrt))
                # scale x2T in place (bf16 * f16 -> bf16), split DVE/GpSimd
                steps.append(lambda: nc.vector.tensor_mul(
                    xb[:, 0:2], xb[:, 0:2],
                    bc[:, None, :].to_broadcast((P, 2, w))))
                steps.append(lambda: nc.gpsimd.tensor_mul(
                    xb[:, 2:4], xb[:, 2:4],
                    bc[:, None, :].to_broadcast((P, 2, w))))
                return steps

            # scratch tiles for the ACT/GpSimd square paths (max width)
            wmax = max(BLOCKS)
            warm2 = sqp.tile([P, wmax], F16, tag="warm2", name="warm2")
            t1b = sqp.tile([P, wmax], F16, tag="t1b", name="t1b")

            # ---- main GEMM m-group ---------------------------------------
            def gemm_mgroup(b, m):
                w = BLOCKS[b]
                c0 = block_c0[b]
                nch = w // 512
                pss = [psp.tile([P, 512], F32, tag="ps",
                                name=f"ps_{b}_{m}_{j}")
                       for j in range(nch)]
                for k in range(kc):
                    lhs = x1T[:, k, m * P : (m + 1) * P]
                    for j in range(nch):
                        nc.tensor.matmul(
                            pss[j][:],
                            lhsT=lhs,
                            rhs=x2S[b][:, k, j * 512 : (j + 1) * 512],
                            start=(k == 0), stop=(k == kc - 1),
                        )
                ot = outp.tile([P, w], BF16, tag="ot", name=f"ot_{b}_{m}")
                sc = inv1[:, m : m + 1]
                for j in range(nch):
                    dst = ot[:, j * 512 : (j + 1) * 512]
                    if j % 2 == 0:
                        nc.vector.tensor_scalar_mul(dst, pss[j][:], sc)
                    else:
                        nc.scalar.activation(
                            dst, pss[j][:],
                            mybir.ActivationFunctionType.Copy, scale=sc,
                        )
                nc.sync.dma_start(
                    out[m * P : (m + 1) * P, c0 : c0 + w], ot[:]
                )

            # head: block0's x2t DMA first (on the otherwise-idle ACT
            # queue, parallel to x1T on sync), then warm the ScalarE
            # activation tables (Square, Sqrt) while it transfers, then the
            # rest of block0 prep and the x1 stats
            b0_steps = prep_block_steps(0, dma_engine=nc.scalar)
            b0_steps[0]()
            nc.scalar.activation(warm[:], ones128[:, 0:1],
                                 mybir.ActivationFunctionType.Square)
            nc.scalar.activation(warm[:], warm[:],
                                 mybir.ActivationFunctionType.Sqrt)
            for step in b0_steps[1:]:
                step()
            prep_x1()
            for b in range(len(BLOCKS)):
                next_steps = (prep_block_steps(b + 1)
                              if b + 1 < len(BLOCKS) else [])
                si = 0
                for m in range(m_tiles):
                    gemm_mgroup(b, m)
                    want = ((m + 1) * len(next_steps) + m_tiles - 1) // m_tiles
                    while si < min(want, len(next_steps)):
                        next_steps[si]()
                        si += 1

    nc.compile()
    return nc


def _get_program():
    key = "default"
    if key not in _PROGRAM_CACHE:
        _PROGRAM_CACHE[key] = build_program()
    return _PROGRAM_CACHE[key]


def make_in_maps(x1: np.ndarray, x2: np.ndarray) -> list:
    x1 = np.asarray(x1, dtype=np.float32)
    x2 = np.asarray(x2, dtype=np.float32)
    assert x1.shape == (N1, D) and x2.shape == (N2, D), (x1.shape, x2.shape)
    x1_b = x1.astype(ml_dtypes.bfloat16)
    x2_b = x2.astype(ml_dtypes.bfloat16)
    x2t_b = np.ascontiguousarray(x2_b.T)
    rows = N1 // N_CORES
    return [
        {
            "x1": np.ascontiguousarray(x1_b[c * rows : (c + 1) * rows]),
            "x1t": np.ascontiguousarray(x1_b[c * rows : (c + 1) * rows].T),
            "x2t": x2t_b,
        }
        for c in range(N_CORES)
    ]


def kernel(x1: np.ndarray, x2: np.ndarray) -> np.ndarray:
    nc = _get_program()
    in_maps = make_in_maps(x1, x2)
    res = run_bass_kernel_spmd(nc, in_maps, core_ids=list(range(N_CORES)))
    return np.concatenate(
        [res.results[c]["out"].astype(np.float32) for c in range(N_CORES)],
        axis=0,
    )


if __name__ == "__main__":
    rng = np.random.default_rng(0)
    a = rng.standard_normal((N1, D), dtype=np.float32)
    b = rng.standard_normal((N2, D), dtype=np.float32)
    got = kernel(a, b)
    n1 = np.maximum(np.linalg.norm(a, axis=-1, keepdims=True), EPS)
    n2 = np.maximum(np.linalg.norm(b, axis=-1, keepdims=True), EPS)
    want = SCALE * (a / n1) @ (b / n2).T
    err = np.abs(got - want)
    rel = np.linalg.norm(got - want) / np.linalg.norm(want)
    print(f"max abs err: {err.max():.3e}  rel: {rel:.3e}")


# revision 14
# speedup vs baseline: 1.4347x; 1.4347x over previous
"""Pairwise cosine-similarity kernel for Trainium2 (8 NeuronCores, SPMD).

Computes out = 16 * normalize(x1) @ normalize(x2).T for x1, x2 [8192, 512] f32.

Sharding: x1 rows are split across the 8 cores (1024 rows each); x2 is
replicated. Each core computes its [1024, 8192] slice of the output; the host
concatenates the slices.

Host-side prep is layout/dtype only: inputs are cast to bf16; x1 and x2 are
additionally shipped pre-transposed so neither GEMM operand needs on-device
transposition. All FLOPs (norms, normalization, GEMM, scaling) run on device:

  1. x1 row norms from the natural-layout bf16 shard: sum-of-squares split
     between ScalarE (Square+accum) and DVE (tensor_tensor_reduce), sqrt,
     clamp, reciprocal -> inv1 [128, 8] compact (col m <-> output row-tile
     m). The 16/n1 scale is folded into the PSUM->SBUF copy of each output
     tile (per-partition scale), so x1 goes stationary into the PE untouched.
  2. x2 in column blocks: row norms from the natural-layout copy (same
     ACT/DVE split, compact [128, 4] per 512-row group -> tiny sqrt/recip),
     partition-broadcast via ones-matmul against diagonalized inverse norms
     (bf16), then the pre-transposed x2T block is scaled in place (16-bit
     DVE tensor_tensor).
  3. Main GEMM per (block, m): psum[128, 1024 (2 banks)] += x1T_k.T @ x2S
     over 4 K-chunks with up to 4 moving tiles per stationary load (weight
     reuse window for LDWEIGHTS prefetch), bf16 in, f32 PSUM. Double-width
     PSUM->SBUF copies alternate DVE/ACT with the x1 row-scale fused; the
     SBUF staging tile is bf16 (halves output DMA); host casts back to f32.

Next-block prep is emitted interleaved and front-loaded between the current
block's m-groups (per-engine queues are strict FIFO; bulk-emitting prep
ahead of pending PSUM copies would stall the copies behind it, and late
prep stalls the next block's matmuls).
"""

import sys

for _p in ("/root/.axon_site/_ro/trn_rl_repo", "/opt/trn_rl_repo"):
    if _p not in sys.path:
        sys.path.append(_p)

import ml_dtypes
import numpy as np

import concourse.bass as bass
import concourse.tile as tile
from concourse import bacc, mybir
from concourse.bass_utils import run_bass_kernel_spmd
from concourse.masks import make_identity

F32 = mybir.dt.float32
BF16 = mybir.dt.bfloat16
P = 128
SCALE = 16.0
EPS = 1e-8

N_CORES = 8
N1 = 8192  # x1 rows (total)
N2 = 8192  # x2 rows
D = 512  # feature dim

# x2 column blocks (rows of x2 == columns of out). Lead/tail blocks are
# small so the first GEMM starts early and the drain is short; middle
# blocks are 2048 (4 PSUM banks).
BLOCKS = (512, 1024, 2048, 2048, 2048, 512)
assert sum(BLOCKS) == N2

_PROGRAM_CACHE = {}


def build_program(n1_local=N1 // N_CORES, n2=N2, d=D):
    kc = d // P  # K-chunks of the contraction dim (4)
    m_tiles = n1_local // P  # x1 row-tiles per core (8)

    nc = bacc.Bacc("TRN2", target_bir_lowering=False, debug=False,
                   num_devices=N_CORES)
    x1 = nc.dram_tensor("x1", [n1_local, d], BF16, kind="ExternalInput")
    x1t = nc.dram_tensor("x1t", [d, n1_local], BF16, kind="ExternalInput")
    x2n = nc.dram_tensor("x2n", [n2, d], BF16, kind="ExternalInput")
    x2t = nc.dram_tensor("x2t", [d, n2], BF16, kind="ExternalInput")
    out = nc.dram_tensor("out", [n1_local, n2], BF16, kind="ExternalOutput")

    with tile.TileContext(nc) as tc:
        with (
            tc.tile_pool(name="const", bufs=1) as const,
            tc.tile_pool(name="ld", bufs=3) as ld,
            tc.tile_pool(name="sq", bufs=2) as sqp,
            tc.tile_pool(name="stat", bufs=4) as stat,
            tc.tile_pool(name="xt", bufs=1) as xt,
            tc.tile_pool(name="bc", bufs=2) as bcp,
            tc.tile_pool(name="outp", bufs=3) as outp,
            tc.tile_pool(name="ps", bufs=8, space="PSUM") as psp,
        ):
            ones128 = const.tile([P, P], BF16)
            nc.gpsimd.memset(ones128[:], 1.0)
            ident4 = const.tile([P, 4, P], BF16)
            nc.gpsimd.memset(ident4[:], 0.0)
            for b in range(4):
                make_identity(nc, ident4[:, b], nomemset=True)
            warm = stat.tile([P, 1], F32, tag="warm")

            x1r = x1.ap().rearrange("(g j p) e -> g p j e", j=4, p=P)
            x2r = x2n.ap().rearrange("(g j p) e -> g p j e", j=4, p=P)
            x2t_r = x2t.ap().rearrange("(k p) n -> p k n", p=P)

            # stationary operand: x1T [128, kc, n1_local]
            x1T = xt.tile([P, kc, n1_local], BF16, name="x1T")
            nc.sync.dma_start(
                x1T[:], x1t.ap().rearrange("(k p) n -> p k n", p=P)
            )

            # moving operand per block [128, kc, w], scaled in place
            x2S = [xt.tile([P, kc, w], BF16, tag=f"x2S_{b}", name=f"x2S_{b}")
                   for b, w in enumerate(BLOCKS)]

            def row_stats(src_r, g, inv_dst, scale_const, eps, dma_eng):
                """inv_dst [P, 4] = scale / row_norm for rows g*512..g*512+511
                (col j <-> rows g*512 + j*128 + p). Sum-of-squares is split
                ACT (j=0,1) / DVE tensor_tensor_reduce (j=2,3)."""
                ld_t = ld.tile([P, 4, d], BF16, tag="ld")
                dma_eng.dma_start(ld_t[:], src_r[g])
                ssq = stat.tile([P, 4], F32, tag="ssq")
                for j in range(4):
                    sq_t = sqp.tile([P, d], BF16, tag="sq")
                    nc.scalar.activation(
                        sq_t[:], ld_t[:, j],
                        mybir.ActivationFunctionType.Square,
                        accum_out=ssq[:, j : j + 1],
                    )
                nrm = stat.tile([P, 4], F32, tag="nrm")
                nc.scalar.activation(
                    nrm[:], ssq[:], mybir.ActivationFunctionType.Sqrt
                )
                if eps:
                    nc.vector.tensor_scalar_max(nrm[:], nrm[:], EPS)
                nc.vector.reciprocal(inv_dst, nrm[:])
                if scale_const != 1.0:
                    nc.vector.tensor_scalar_mul(inv_dst, inv_dst, scale_const)

            # ---- x2 per block: dma -> stats -> bcast -> scale -----------
            block_c0 = [sum(BLOCKS[:b]) for b in range(len(BLOCKS))]

            def prep_block_steps(b, x2t_eng=None, ld_eng=None):
                w = BLOCKS[b]
                c0 = block_c0[b]
                rt = w // P
                xb = x2S[b]
                inv2 = stat.tile([P, rt], F32, tag=f"inv2_{b % 2}",
                                 name=f"inv2_{b}")
                bc = bcp.tile([P, w], BF16, tag="bc", name=f"bc_{b}")
                steps = [lambda: (x2t_eng or nc.sync).dma_start(
                    xb[:], x2t_r[:, :, c0 : c0 + w])]
                for g2 in range(rt // 4):
                    steps.append(lambda g2=g2: row_stats(
                        x2r, c0 // 512 + g2,
                        inv2[:, g2 * 4 : (g2 + 1) * 4], 1.0, False,
                        ld_eng or nc.gpsimd))

                def bcast(c0i):
                    dg4 = stat.tile([P, 4, P], BF16, tag="dg4",
                                    name=f"dg4_{b}_{c0i}")
                    nc.vector.tensor_mul(
                        dg4[:], ident4[:],
                        inv2[:, c0i : c0i + 4, None].to_broadcast((P, 4, P)),
                    )
                    ps_b = psp.tile([P, 512], F32, tag="ps",
                                    name=f"psb_{b}_{c0i}")
                    nc.tensor.matmul(ps_b[:], lhsT=ones128[:], rhs=dg4[:],
                                     start=True, stop=True)
                    nc.vector.tensor_copy(
                        bc[:, c0i * P : (c0i + 4) * P], ps_b[:]
                    )

                for c0i in range(0, rt, 4):
                    steps.append(lambda c0i=c0i: bcast(c0i))
                # scale x2T in place (bf16 * bf16 -> bf16, 16-bit DVE rate)
                steps.append(lambda: nc.vector.tensor_mul(
                    xb[:, 0:2], xb[:, 0:2],
                    bc[:, None, :].to_broadcast((P, 2, w))))
                steps.append(lambda: nc.vector.tensor_mul(
                    xb[:, 2:4], xb[:, 2:4],
                    bc[:, None, :].to_broadcast((P, 2, w))))
                return steps

            # ---- x1 stats: compact inverse norms, scaled by 16 ----------
            inv1 = stat.tile([P, m_tiles], F32, name="inv1")

            def prep_x1():
                for g in range(m_tiles // 4):
                    row_stats(x1r, g, inv1[:, g * 4 : (g + 1) * 4], SCALE,
                              True, nc.gpsimd)

            # ---- main GEMM m-group --------------------------------------
            def gemm_mgroup(b, m):
                w = BLOCKS[b]
                c0 = block_c0[b]
                nch = w // 512
                pss = [psp.tile([P, 512], F32, tag="ps",
                                name=f"ps_{b}_{m}_{j}")
                       for j in range(nch)]
                for k in range(kc):
                    lhs = x1T[:, k, m * P : (m + 1) * P]
                    for j in range(nch):
                        nc.tensor.matmul(
                            pss[j][:],
                            lhsT=lhs,
                            rhs=x2S[b][:, k, j * 512 : (j + 1) * 512],
                            start=(k == 0), stop=(k == kc - 1),
                        )
                ot = outp.tile([P, w], BF16, tag="ot", name=f"ot_{b}_{m}")
                sc = inv1[:, m : m + 1]
                for j in range(nch):
                    dst = ot[:, j * 512 : (j + 1) * 512]
                    if (j + m) % 2 == 0:
                        nc.vector.tensor_scalar_mul(dst, pss[j][:], sc)
                    else:
                        nc.scalar.activation(
                            dst, pss[j][:],
                            mybir.ActivationFunctionType.Copy, scale=sc,
                        )
                nc.sync.dma_start(
                    out[m * P : (m + 1) * P, c0 : c0 + w], ot[:]
                )

            # head: block0's DMAs first (x2t on ACT queue, x2n rows on
            # GpSimd, both parallel to x1T on sync), table warmups during
            # the transfers, then the rest of block0 prep and x1 stats
            b0_steps = prep_block_steps(0, x2t_eng=nc.scalar,
                                        ld_eng=nc.gpsimd)
            b0_steps[0]()
            nc.scalar.activation(warm[:], ones128[:, 0:1],
                                 mybir.ActivationFunctionType.Square)
            nc.scalar.activation(warm[:], warm[:],
                                 mybir.ActivationFunctionType.Sqrt)
            for step in b0_steps[1:]:
                step()
            prep_x1()
            for b in range(len(BLOCKS)):
                next_steps = (prep_block_steps(b + 1)
                              if b + 1 < len(BLOCKS) else [])
                si = 0
                for m in range(m_tiles):
                    gemm_mgroup(b, m)
                    # front-load: all prep emitted by m-group 5
                    want = ((m + 1) * len(next_steps) + 5) // 6
                    while si < min(want, len(next_steps)):
                        next_steps[si]()
                        si += 1

    nc.compile()
    return nc


def _get_program():
    key = "default"
    if key not in _PROGRAM_CACHE:
        _PROGRAM_CACHE[key] = build_program()
    return _PROGRAM_CACHE[key]


def make_in_maps(x1: np.ndarray, x2: np.ndarray) -> list:
    x1 = np.asarray(x1, dtype=np.float32)
    x2 = np.asarray(x2, dtype=np.float32)
    assert x1.shape == (N1, D) and x2.shape == (N2, D), (x1.shape, x2.shape)
    x1_b = x1.astype(ml_dtypes.bfloat16)
    x2_b = x2.astype(ml_dtypes.bfloat16)
    x2t_b = np.ascontiguousarray(x2_b.T)
    rows = N1 // N_CORES
    return [
        {
            "x1": np.ascontiguousarray(x1_b[c * rows : (c + 1) * rows]),
            "x1t": np.ascontiguousarray(x1_b[c * rows : (c + 1) * rows].T),
            "x2n": x2_b,
            "x2t": x2t_b,
        }
        for c in range(N_CORES)
    ]


def kernel(x1: np.ndarray, x2: np.ndarray) -> np.ndarray:
    nc = _get_program()
    in_maps = make_in_maps(x1, x2)
    res = run_bass_kernel_spmd(nc, in_maps, core_ids=list(range(N_CORES)))
    return np.concatenate(
        [res.results[c]["out"].astype(np.float32) for c in range(N_CORES)],
        axis=0,
    )


if __name__ == "__main__":
    rng = np.random.default_rng(0)
    a = rng.standard_normal((N1, D), dtype=np.float32)
    b = rng.standard_normal((N2, D), dtype=np.float32)
    got = kernel(a, b)
    n1 = np.maximum(np.linalg.norm(a, axis=-1, keepdims=True), EPS)
    n2 = np.maximum(np.linalg.norm(b, axis=-1, keepdims=True), EPS)
    want = SCALE * (a / n1) @ (b / n2).T
    err = np.abs(got - want)
    rel = np.linalg.norm(got - want) / np.linalg.norm(want)
    print(f"max abs err: {err.max():.3e}  rel: {rel:.3e}")
